# revision 14
# baseline (speedup 1.0000x reference)
"""Distributed GQA attention kernel for one TRN2 chip (8 NeuronCores).

Problem: B=2, L=2048, HID=2048, H=32 q-heads, HKV=8 kv-heads, D=64,
rotary embedding, causal softmax, o-proj.

Sharding: core i -> batch b=i//4, TP rank r=i%4.  Each core computes
8 q-heads / 2 kv-heads of its batch, all-gathers the attention outputs
(feature-major, bf16) within its 4-core TP group, then computes its
512 output columns of the o-proj.  Host assembles the full output.

All matmuls run in bf16 with fp32 PSUM accumulation.  Softmax skips the
row-max (logits are bounded ~|6| for these input scales) and obtains
row sums for free by appending a 64-wide ones block to V's stationary
operand; normalization is a DVE reciprocal + multiply.
"""

import sys

sys.path.insert(0, "/opt/trn_rl_repo")

import numpy as np
import ml_dtypes

B, L, HID = 2, 2048, 2048
H, HKV, D = 32, 8, 64
N_CORES = 8
TP = 4           # tensor-parallel group size
HL = 8           # q heads per core
CW = 512         # o-proj output columns per core
TT = 4           # t tiles of 512 over L
CCH = HID // 128 # contraction chunks (16)
BF16 = ml_dtypes.bfloat16

_cache = {}


def _build_graph(dbg=None):
    import concourse.bass as bass
    import concourse.tile as tile
    from concourse import bacc, mybir

    dt = mybir.dt
    f32, bf16 = dt.float32, dt.bfloat16

    nc = bacc.Bacc("TRN2", target_bir_lowering=False, debug=False,
                   num_devices=N_CORES)

    xT = nc.dram_tensor("xT", [HID, L], bf16, kind="ExternalInput")
    WqT = nc.dram_tensor("WqT", [HID, HL * D], bf16, kind="ExternalInput")
    WkT = nc.dram_tensor("WkT", [HID, 128], bf16, kind="ExternalInput")
    WvT = nc.dram_tensor("WvT", [HID, 128], bf16, kind="ExternalInput")
    WoT = nc.dram_tensor("WoT", [HID, CW], bf16, kind="ExternalInput")
    C1q = nc.dram_tensor("C1q", [128, L], bf16, kind="ExternalInput")
    C2q = nc.dram_tensor("C2q", [128, L], bf16, kind="ExternalInput")
    C1k = nc.dram_tensor("C1k", [128, L], bf16, kind="ExternalInput")
    C2k = nc.dram_tensor("C2k", [128, L], bf16, kind="ExternalInput")
    out = nc.dram_tensor("out", [CW, L], f32, kind="ExternalOutput")
    dbg_shapes = {"qq": [128, HL // 2 * L], "kk": [128, L],
                  "v2": [128, CCH * 256], "ao": [128, HL // 2 * L],
                  "gathered": [TP * TP * 128, L], "bounce": [TP * 128, L]}
    dbg_t = (nc.dram_tensor("dbg", dbg_shapes[dbg], bf16,
                            kind="ExternalOutput") if dbg else None)

    def bcast_m(ap2d, n):
        # [P, F] -> [P, n, F] with a step-0 middle dim (free-dim broadcast)
        return bass.AP(ap2d.tensor, ap2d.offset,
                       [ap2d.ap[0], [0, n], ap2d.ap[1]])

    with tile.TileContext(nc) as tc:
        with (
            tc.tile_pool(name="persist", bufs=1) as persist,
            tc.tile_pool(name="ps", bufs=2, space="PSUM") as ps,
            tc.tile_pool(name="psbig", bufs=2, space="PSUM") as psbig,
            tc.tile_pool(name="pp", bufs=3) as pp,
            tc.tile_pool(name="dram", bufs=1, space="DRAM") as dram,
        ):
            # ---- persistent SBUF tensors ----
            qq = persist.tile([128, HL // 2 * L], bf16)      # roped Q^T, 2MB
            kk = persist.tile([128, L], bf16)                # roped K^T (2 kv)
            v2t = persist.tile([128, L], bf16)               # V^T staging
            v2 = persist.tile([128, CCH * 256], bf16)        # [V|1|V|1] per kt
            ao = persist.tile([128, HL // 2 * L], bf16)      # attn out^T
            c1q = persist.tile([128, L], bf16)
            c2q = persist.tile([128, L], bf16)
            c1k = persist.tile([128, L], bf16)
            c2k = persist.tile([128, L], bf16)
            for t_sb, t_dr in ((c1q, C1q), (c2q, C2q), (c1k, C1k), (c2k, C2k)):
                nc.sync.dma_start(t_sb[:], t_dr[:])

            # ones blocks of v2 (columns 64:128 and 192:256 of each kt group)
            for off in (64, 192):
                ones_view = bass.AP(v2.tensor, v2.offset + off,
                                    [v2.ap[0], [256, CCH], [1, 64]])
                nc.gpsimd.memset(ones_view, 1.0)

            # =========== Phase B: projections + RoPE ===========
            with (
                tc.tile_pool(name="wsb", bufs=1) as wsb,
                tc.tile_pool(name="xt", bufs=2) as xtp,
                tc.tile_pool(name="rope", bufs=2) as rope,
            ):
                wq_sb = wsb.tile([128, CCH * 512], bf16)
                nc.sync.dma_start(
                    wq_sb[:].rearrange("p (c m) -> p c m", m=512),
                    WqT[:].rearrange("(c p) m -> p c m", p=128))
                wk_sb = wsb.tile([128, CCH * 128], bf16)
                nc.sync.dma_start(
                    wk_sb[:].rearrange("p (c m) -> p c m", m=128),
                    WkT[:].rearrange("(c p) m -> p c m", p=128))
                wv_sb = wsb.tile([128, CCH * 128], bf16)
                nc.sync.dma_start(
                    wv_sb[:].rearrange("p (c m) -> p c m", m=128),
                    WvT[:].rearrange("(c p) m -> p c m", p=128))

                xT_view = xT[:].rearrange("(c p) t -> p c t", p=128)

                for tt in range(TT):
                    ts = slice(tt * 512, (tt + 1) * 512)
                    xt = xtp.tile([128, CCH * 512], bf16, tag="xt")
                    nc.sync.dma_start(
                        xt[:].rearrange("p (c t) -> p c t", t=512),
                        xT_view[:, :, ts])

                    # --- Q: 4 M-tiles (head pair (jj, jj+4) each) ---
                    qraw = rope.tile([128, 4 * 512], bf16, tag="qraw")
                    for m in range(4):
                        psq = ps.tile([128, 512], f32, tag="mm")
                        for c in range(CCH):
                            nc.tensor.matmul(
                                psq[:],
                                lhsT=wq_sb[:, c * 512 + m * 128:
                                           c * 512 + (m + 1) * 128],
                                rhs=xt[:, c * 512:(c + 1) * 512],
                                start=(c == 0), stop=(c == CCH - 1))
                        nc.scalar.copy(qraw[:, m * 512:(m + 1) * 512], psq[:])

                    # --- K ---
                    kraw = rope.tile([128, 512], bf16, tag="kraw")
                    psk = ps.tile([128, 512], f32, tag="mm")
                    for c in range(CCH):
                        nc.tensor.matmul(
                            psk[:], lhsT=wk_sb[:, c * 128:(c + 1) * 128],
                            rhs=xt[:, c * 512:(c + 1) * 512],
                            start=(c == 0), stop=(c == CCH - 1))
                    nc.scalar.copy(kraw[:], psk[:])

                    # --- V ---
                    psv = ps.tile([128, 512], f32, tag="mm")
                    for c in range(CCH):
                        nc.tensor.matmul(
                            psv[:], lhsT=wv_sb[:, c * 128:(c + 1) * 128],
                            rhs=xt[:, c * 512:(c + 1) * 512],
                            start=(c == 0), stop=(c == CCH - 1))
                    nc.scalar.copy(v2t[:, ts], psv[:])

                    # --- RoPE on Q ---
                    qsw = rope.tile([128, 4 * 512], bf16, tag="qsw")
                    for a, b_ in ((0, 32), (32, 0), (64, 96), (96, 64)):
                        nc.sync.dma_start(qsw[b_:b_ + 32, :], qraw[a:a + 32, :])
                    qtmp = rope.tile([128, 4 * 512], bf16, tag="qtmp")
                    q3 = qraw[:].rearrange("p (m t) -> p m t", t=512)
                    s3 = qsw[:].rearrange("p (m t) -> p m t", t=512)
                    t3 = qtmp[:].rearrange("p (m t) -> p m t", t=512)
                    nc.vector.tensor_tensor(t3, q3, bcast_m(c1q[:, ts], 4),
                                            mybir.AluOpType.mult)
                    nc.vector.tensor_tensor(s3, s3, bcast_m(c2q[:, ts], 4),
                                            mybir.AluOpType.mult)
                    qqd = bass.AP(qq.tensor, qq.offset + tt * 512,
                                  [qq.ap[0], [2048, 4], [1, 512]])
                    nc.vector.tensor_tensor(qqd, t3, s3, mybir.AluOpType.add)

                    # --- RoPE on K ---
                    ksw = rope.tile([128, 512], bf16, tag="ksw")
                    for a, b_ in ((0, 32), (32, 0), (64, 96), (96, 64)):
                        nc.sync.dma_start(ksw[b_:b_ + 32, :], kraw[a:a + 32, :])
                    ktmp = rope.tile([128, 512], bf16, tag="ktmp")
                    nc.vector.tensor_tensor(ktmp[:], kraw[:], c1k[:, ts],
                                            mybir.AluOpType.mult)
                    nc.vector.tensor_tensor(ksw[:], ksw[:], c2k[:, ts],
                                            mybir.AluOpType.mult)
                    nc.vector.tensor_tensor(kk[:, ts], ktmp[:], ksw[:],
                                            mybir.AluOpType.add)

                # --- V transpose to token-major (into v2, cols 0:64/128:192)
                for g in range(2):
                    v2_dst = bass.AP(v2.tensor, v2.offset + g * 128,
                                     [v2.ap[0], [256, CCH], [1, 64]])
                    nc.sync.dma_start_transpose(v2_dst, v2t[g * 64:(g + 1) * 64, :])

            # =========== Phase C: attention ===========
            for jj in range(4):
                qoff = jj * L
                for qT in range(4):
                    nkt = 4 * qT + 4
                    qs = slice(qoff + qT * 512, qoff + (qT + 1) * 512)
                    o0 = ps.tile([128, 512], f32, tag="o")
                    o1 = ps.tile([128, 512], f32, tag="o")
                    for kp in range(nkt // 2):
                        sb0 = psbig.tile([128, 1024], f32, tag="s")
                        sb1 = psbig.tile([128, 1024], f32, tag="s")
                        for h in range(2):
                            kt = 2 * kp + h
                            kslc = slice(kt * 128, (kt + 1) * 128)
                            hs = slice(h * 512, (h + 1) * 512)
                            nc.tensor.matmul(
                                sb0[:, hs], lhsT=kk[0:64, kslc],
                                rhs=qq[0:64, qs], start=True, stop=True,
                                tile_position=(0, 0))
                            nc.tensor.matmul(
                                sb1[:, hs], lhsT=kk[64:128, kslc],
                                rhs=qq[64:128, qs], start=True, stop=True,
                                tile_position=(64, 0))
                        p0 = pp.tile([128, 1024], bf16, tag="p")
                        p1 = pp.tile([128, 1024], bf16, tag="p")
                        nc.scalar.activation(p0[:], sb0[:],
                                             mybir.ActivationFunctionType.Exp)
                        nc.scalar.activation(p1[:], sb1[:],
                                             mybir.ActivationFunctionType.Exp)
                        for h in range(2):
                            kt = 2 * kp + h
                            if kt >= 4 * qT:  # diagonal/above: causal mask
                                hs = slice(h * 512, (h + 1) * 512)
                                for p_ in (p0, p1):
                                    nc.gpsimd.affine_select(
                                        out=p_[:, hs], in_=p_[:, hs],
                                        pattern=[[1, 512]],
                                        compare_op=mybir.AluOpType.is_ge,
                                        fill=0.0,
                                        base=qT * 512 - kt * 128,
                                        channel_multiplier=-1)
                        for h in range(2):
                            kt = 2 * kp + h
                            hs = slice(h * 512, (h + 1) * 512)
                            nc.tensor.matmul(
                                o0[:], lhsT=v2[:, kt * 256:kt * 256 + 128],
                                rhs=p0[:, hs], start=(kt == 0),
                                stop=(kt == nkt - 1))
                            nc.tensor.matmul(
                                o1[:], lhsT=v2[:, kt * 256 + 128:(kt + 1) * 256],
                                rhs=p1[:, hs], start=(kt == 0),
                                stop=(kt == nkt - 1))
                    # normalize: rows 0:64 = O^T unnormalized, 64:128 = sums
                    rc = pp.tile([128, 512], f32, tag="rc", bufs=2)
                    nc.vector.reciprocal(rc[64:128, :], o0[64:128, :])
                    nc.vector.tensor_tensor(
                        ao[0:64, qs], o0[0:64, :], rc[64:128, :],
                        mybir.AluOpType.mult)
                    rc2 = pp.tile([128, 512], f32, tag="rc", bufs=2)
                    nc.vector.reciprocal(rc2[64:128, :], o1[64:128, :])
                    nc.vector.tensor_tensor(
                        ao[64:128, qs], o1[0:64, :], rc2[64:128, :],
                        mybir.AluOpType.mult)

            # =========== Phase D: AllGather + o-proj ===========
            bounce = dram.tile([TP * 128, L], bf16)
            gathered = dram.tile([TP * TP * 128, L], bf16)
            # ao rows (g*64+d), free (jj*L + t) -> bounce row 256g+64jj+d
            for g in range(2):
                for jj in range(4):
                    nc.sync.dma_start(
                        bounce[256 * g + 64 * jj:256 * g + 64 * jj + 64, :],
                        ao[g * 64:(g + 1) * 64, jj * L:(jj + 1) * L])
            nc.gpsimd.collective_compute(
                "AllGather", mybir.AluOpType.bypass,
                replica_groups=[[0, 1, 2, 3], [4, 5, 6, 7]],
                ins=[bounce.opt()], outs=[gathered.opt()])

            with (
                tc.tile_pool(name="wo", bufs=1) as wop,
                tc.tile_pool(name="aog", bufs=24) as aogp,
                tc.tile_pool(name="ost", bufs=2) as ostp,
            ):
                wo_sb = wop.tile([128, CCH * 512], bf16)
                nc.sync.dma_start(
                    wo_sb[:].rearrange("p (c m) -> p c m", m=512),
                    WoT[:].rearrange("(c p) m -> p c m", p=128))
                for tt in range(TT):
                    ts = slice(tt * 512, (tt + 1) * 512)
                    aoks = []
                    for c in range(CCH):
                        aok = aogp.tile([128, 512], bf16, tag="aok",
                                        name=f"aok{tt}_{c}")
                        nc.sync.dma_start(aok[:],
                                          gathered[c * 128:(c + 1) * 128, ts])
                        aoks.append(aok)
                    for ct in range(4):
                        pso = ps.tile([128, 512], f32, tag="mm")
                        for c in range(CCH):
                            nc.tensor.matmul(
                                pso[:],
                                lhsT=wo_sb[:, c * 512 + ct * 128:
                                           c * 512 + (ct + 1) * 128],
                                rhs=aoks[c][:], start=(c == 0),
                                stop=(c == CCH - 1))
                        ost = ostp.tile([128, 512], f32, tag="ost")
                        nc.scalar.copy(ost[:], pso[:])
                        nc.sync.dma_start(
                            out[ct * 128:(ct + 1) * 128, ts], ost[:])

            if dbg:
                src = {"qq": qq, "kk": kk, "v2": v2, "ao": ao,
                       "gathered": gathered, "bounce": bounce}[dbg]
                nc.sync.dma_start(dbg_t[:], src[:])

    nc.compile()
    return nc


def _host_prep(hidden_states, cos, sin, Wq, Wk, Wv, Wo):
    """Build the 8 per-core input maps (all host-side slicing/transposes)."""
    scale = float(D) ** -0.5
    # rope coefficient tables [128, L]: 4 groups of 32 rows (d 0:32 pattern)
    cosT = cos[:, :32].T.astype(np.float32)          # [32, L]
    sinT = sin[:, :32].T.astype(np.float32)
    c1 = np.tile(cosT, (4, 1))                       # [128, L]
    c2 = np.concatenate([-sinT, sinT, -sinT, sinT], axis=0)
    tables = {
        "C1q": (c1 * scale).astype(BF16), "C2q": (c2 * scale).astype(BF16),
        "C1k": c1.astype(BF16), "C2k": c2.astype(BF16),
    }
    xTb = [np.ascontiguousarray(hidden_states[b].T).astype(BF16)
           for b in range(B)]
    in_maps = []
    for i in range(N_CORES):
        b, r = divmod(i, TP)
        # Wq rows reordered: M-tile m = heads (8r+m, 8r+4+m)
        rows = []
        for m in range(4):
            rows.append(Wq[(8 * r + m) * D:(8 * r + m + 1) * D])
            rows.append(Wq[(8 * r + 4 + m) * D:(8 * r + 4 + m + 1) * D])
        WqT_i = np.ascontiguousarray(np.concatenate(rows, 0).T).astype(BF16)
        WkT_i = np.ascontiguousarray(
            Wk[2 * r * D:(2 * r + 2) * D].T).astype(BF16)
        WvT_i = np.ascontiguousarray(
            Wv[2 * r * D:(2 * r + 2) * D].T).astype(BF16)
        WoT_i = np.ascontiguousarray(
            Wo[CW * r:CW * (r + 1), :].T).astype(BF16)
        in_maps.append({
            "xT": xTb[b], "WqT": WqT_i, "WkT": WkT_i, "WvT": WvT_i,
            "WoT": WoT_i, **tables,
        })
    return in_maps


def kernel(hidden_states, cos, sin, Wq, Wk, Wv, Wo, _want_profile=False):
    from concourse.bass_utils import run_bass_kernel_spmd

    if "nc" not in _cache:
        _cache["nc"] = _build_graph()
    nc = _cache["nc"]
    in_maps = _host_prep(np.asarray(hidden_states), np.asarray(cos),
                         np.asarray(sin), np.asarray(Wq), np.asarray(Wk),
                         np.asarray(Wv), np.asarray(Wo))
    res = run_bass_kernel_spmd(nc, in_maps, list(range(N_CORES)),
                               trace=_want_profile)
    # assemble: core (b, r) holds out^T [512, L] = cols [512r, 512r+512) of b
    full = np.empty((B, L, HID), np.float32)
    for i in range(N_CORES):
        b, r = divmod(i, TP)
        full[b, :, CW * r:CW * (r + 1)] = res.results[i]["out"].T
    if _want_profile:
        return full, res
    return full


# revision 21
# speedup vs baseline: 1.3551x; 1.3551x over previous
"""Distributed GQA attention kernel for one TRN2 chip (8 NeuronCores).

Problem: B=2, L=2048, HID=2048, H=32 q-heads, HKV=8 kv-heads, D=64,
rotary embedding, causal softmax, o-proj.

Sharding: core i -> batch b=i//4, TP rank r=i%4.  Each core computes
8 q-heads / 2 kv-heads of its batch, all-gathers the attention outputs
(feature-major, bf16) within its 4-core TP group, then computes its
512 output columns of the o-proj.  Host assembles the full output.

All matmuls run in bf16 with fp32 PSUM accumulation.  Softmax skips the
row-max (logits are bounded ~|6| for these input scales) and obtains
row sums for free by appending a 64-wide ones block to V's stationary
operand; normalization is a DVE reciprocal + multiply.
"""

import sys

sys.path.insert(0, "/opt/trn_rl_repo")

import numpy as np
import ml_dtypes

B, L, HID = 2, 2048, 2048
H, HKV, D = 32, 8, 64
N_CORES = 8
TP = 4           # tensor-parallel group size
HL = 8           # q heads per core
CW = 512         # o-proj output columns per core
TT = 4           # t tiles of 512 over L
CCH = HID // 128 # contraction chunks (16)
BF16 = ml_dtypes.bfloat16

_cache = {}


def _build_graph(dbg=None):
    import concourse.bass as bass
    import concourse.tile as tile
    from concourse import bacc, mybir

    dt = mybir.dt
    f32, bf16 = dt.float32, dt.bfloat16

    nc = bacc.Bacc("TRN2", target_bir_lowering=False, debug=False,
                   num_devices=N_CORES)

    xT = nc.dram_tensor("xT", [HID, L], bf16, kind="ExternalInput")
    WqT = nc.dram_tensor("WqT", [HID, HL * D], bf16, kind="ExternalInput")
    WkT = nc.dram_tensor("WkT", [HID, 128], bf16, kind="ExternalInput")
    WvT = nc.dram_tensor("WvT", [HID, 128], bf16, kind="ExternalInput")
    WoT = nc.dram_tensor("WoT", [HID, CW], bf16, kind="ExternalInput")
    C1q = nc.dram_tensor("C1q", [128, L], bf16, kind="ExternalInput")
    C2q = nc.dram_tensor("C2q", [128, L], bf16, kind="ExternalInput")
    C1k = nc.dram_tensor("C1k", [128, L], bf16, kind="ExternalInput")
    C2k = nc.dram_tensor("C2k", [128, L], bf16, kind="ExternalInput")
    out = nc.dram_tensor("out", [CW, L], f32, kind="ExternalOutput")
    dbg_shapes = {"qq": [128, HL // 2 * L], "kk": [128, L],
                  "v2": [128, CCH * 256], "ao": [128, HL // 2 * L],
                  "gathered": [TP * TP * 128, L], "bounce": [TP * 128, L]}
    dbg_t = (nc.dram_tensor("dbg", dbg_shapes[dbg], bf16,
                            kind="ExternalOutput") if dbg else None)

    def bcast_m(ap2d, n):
        # [P, F] -> [P, n, F] with a step-0 middle dim (free-dim broadcast)
        return bass.AP(ap2d.tensor, ap2d.offset,
                       [ap2d.ap[0], [0, n], ap2d.ap[1]])

    with tile.TileContext(nc) as tc:
        with (
            tc.tile_pool(name="persist", bufs=1) as persist,
            tc.tile_pool(name="ps", bufs=2, space="PSUM") as ps,
            tc.tile_pool(name="psbig", bufs=2, space="PSUM") as psbig,
            tc.tile_pool(name="pp", bufs=3) as pp,
            tc.tile_pool(name="dram", bufs=1, space="DRAM") as dram,
        ):
            # ---- persistent SBUF tensors ----
            qq = persist.tile([128, HL // 2 * L], bf16)      # roped Q^T, 2MB
            kk = persist.tile([128, L], bf16)                # roped K^T (2 kv)
            v2t = persist.tile([128, L], bf16)               # V^T staging
            v2 = persist.tile([128, CCH * 256], bf16)        # [V|1|V|1] per kt
            ao = persist.tile([128, HL // 2 * L], bf16)      # attn out^T
            c1q = persist.tile([128, L], bf16)
            c2q = persist.tile([128, L], bf16)
            c1k = persist.tile([128, L], bf16)
            c2k = persist.tile([128, L], bf16)
            for t_sb, t_dr in ((c1q, C1q), (c2q, C2q), (c1k, C1k), (c2k, C2k)):
                nc.scalar.dma_start(t_sb[:], t_dr[:])

            # ones blocks of v2 (columns 64:128 and 192:256 of each kt group)
            for off in (64, 192):
                ones_view = bass.AP(v2.tensor, v2.offset + off,
                                    [v2.ap[0], [256, CCH], [1, 64]])
                nc.gpsimd.memset(ones_view, 1.0)

            # =========== Phase B: projections + RoPE ===========
            with (
                tc.tile_pool(name="wsb", bufs=1) as wsb,
                tc.tile_pool(name="xt", bufs=2) as xtp,
                tc.tile_pool(name="rope", bufs=2) as rope,
            ):
                wq_sb = wsb.tile([128, CCH * 512], bf16)
                nc.scalar.dma_start(
                    wq_sb[:].rearrange("p (c m) -> p c m", m=512),
                    WqT[:].rearrange("(c p) m -> p c m", p=128))
                wk_sb = wsb.tile([128, CCH * 128], bf16)
                nc.scalar.dma_start(
                    wk_sb[:].rearrange("p (c m) -> p c m", m=128),
                    WkT[:].rearrange("(c p) m -> p c m", p=128))
                wv_sb = wsb.tile([128, CCH * 128], bf16)
                nc.scalar.dma_start(
                    wv_sb[:].rearrange("p (c m) -> p c m", m=128),
                    WvT[:].rearrange("(c p) m -> p c m", p=128))

                xT_view = xT[:].rearrange("(c p) t -> p c t", p=128)

                for tt in range(TT):
                    ts = slice(tt * 512, (tt + 1) * 512)
                    xt = xtp.tile([128, CCH * 512], bf16, tag="xt")
                    nc.sync.dma_start(
                        xt[:].rearrange("p (c t) -> p c t", t=512),
                        xT_view[:, :, ts])

                    # --- Q: 4 M-tiles (head pair (jj, jj+4) each) ---
                    qraw = rope.tile([128, 4 * 512], bf16, tag="qraw")
                    for m in range(4):
                        psq = ps.tile([128, 512], f32, tag="mm")
                        for c in range(CCH):
                            nc.tensor.matmul(
                                psq[:],
                                lhsT=wq_sb[:, c * 512 + m * 128:
                                           c * 512 + (m + 1) * 128],
                                rhs=xt[:, c * 512:(c + 1) * 512],
                                start=(c == 0), stop=(c == CCH - 1))
                        nc.scalar.copy(qraw[:, m * 512:(m + 1) * 512], psq[:])

                    # --- K ---
                    kraw = rope.tile([128, 512], bf16, tag="kraw")
                    psk = ps.tile([128, 512], f32, tag="mm")
                    for c in range(CCH):
                        nc.tensor.matmul(
                            psk[:], lhsT=wk_sb[:, c * 128:(c + 1) * 128],
                            rhs=xt[:, c * 512:(c + 1) * 512],
                            start=(c == 0), stop=(c == CCH - 1))
                    nc.scalar.copy(kraw[:], psk[:])

                    # --- V ---
                    psv = ps.tile([128, 512], f32, tag="mm")
                    for c in range(CCH):
                        nc.tensor.matmul(
                            psv[:], lhsT=wv_sb[:, c * 128:(c + 1) * 128],
                            rhs=xt[:, c * 512:(c + 1) * 512],
                            start=(c == 0), stop=(c == CCH - 1))
                    nc.scalar.copy(v2t[:, ts], psv[:])

                    # --- RoPE on Q ---
                    qsw = rope.tile([128, 4 * 512], bf16, tag="qsw")
                    for a, b_ in ((0, 32), (32, 0), (64, 96), (96, 64)):
                        nc.scalar.dma_start(qsw[b_:b_ + 32, :], qraw[a:a + 32, :])
                    qtmp = rope.tile([128, 4 * 512], bf16, tag="qtmp")
                    q3 = qraw[:].rearrange("p (m t) -> p m t", t=512)
                    s3 = qsw[:].rearrange("p (m t) -> p m t", t=512)
                    t3 = qtmp[:].rearrange("p (m t) -> p m t", t=512)
                    nc.vector.tensor_tensor(t3, q3, bcast_m(c1q[:, ts], 4),
                                            mybir.AluOpType.mult)
                    nc.vector.tensor_tensor(s3, s3, bcast_m(c2q[:, ts], 4),
                                            mybir.AluOpType.mult)
                    qqd = bass.AP(qq.tensor, qq.offset + tt * 512,
                                  [qq.ap[0], [2048, 4], [1, 512]])
                    nc.vector.tensor_tensor(qqd, t3, s3, mybir.AluOpType.add)

                    # --- RoPE on K ---
                    ksw = rope.tile([128, 512], bf16, tag="ksw")
                    for a, b_ in ((0, 32), (32, 0), (64, 96), (96, 64)):
                        nc.scalar.dma_start(ksw[b_:b_ + 32, :], kraw[a:a + 32, :])
                    ktmp = rope.tile([128, 512], bf16, tag="ktmp")
                    nc.vector.tensor_tensor(ktmp[:], kraw[:], c1k[:, ts],
                                            mybir.AluOpType.mult)
                    nc.vector.tensor_tensor(ksw[:], ksw[:], c2k[:, ts],
                                            mybir.AluOpType.mult)
                    nc.vector.tensor_tensor(kk[:, ts], ktmp[:], ksw[:],
                                            mybir.AluOpType.add)

                # --- V transpose to token-major (into v2, cols 0:64/128:192)
                for g in range(2):
                    v2_dst = bass.AP(v2.tensor, v2.offset + g * 128,
                                     [v2.ap[0], [256, CCH], [1, 64]])
                    nc.sync.dma_start_transpose(v2_dst, v2t[g * 64:(g + 1) * 64, :])

            # =========== Phase C: attention ===========
            for jj in range(4):
                qoff = jj * L
                for qT in range(4):
                    nkt = 4 * qT + 4
                    qs = slice(qoff + qT * 512, qoff + (qT + 1) * 512)
                    o0 = ps.tile([128, 512], f32, tag="o")
                    o1 = ps.tile([128, 512], f32, tag="o")
                    for kp in range(nkt // 2):
                        sb0 = psbig.tile([128, 1024], f32, tag="s")
                        sb1 = psbig.tile([128, 1024], f32, tag="s")
                        for h in range(2):
                            kt = 2 * kp + h
                            kslc = slice(kt * 128, (kt + 1) * 128)
                            hs = slice(h * 512, (h + 1) * 512)
                            nc.tensor.matmul(
                                sb0[:, hs], lhsT=kk[0:64, kslc],
                                rhs=qq[0:64, qs], start=True, stop=True,
                                tile_position=(0, 0))
                            nc.tensor.matmul(
                                sb1[:, hs], lhsT=kk[64:128, kslc],
                                rhs=qq[64:128, qs], start=True, stop=True,
                                tile_position=(64, 0))
                        p0 = pp.tile([128, 1024], bf16, tag="p")
                        p1 = pp.tile([128, 1024], bf16, tag="p")
                        nc.scalar.activation(p0[:], sb0[:],
                                             mybir.ActivationFunctionType.Exp)
                        nc.scalar.activation(p1[:], sb1[:],
                                             mybir.ActivationFunctionType.Exp)
                        for h in range(2):
                            kt = 2 * kp + h
                            if kt >= 4 * qT:  # diagonal/above: causal mask
                                hs = slice(h * 512, (h + 1) * 512)
                                for p_ in (p0, p1):
                                    nc.gpsimd.affine_select(
                                        out=p_[:, hs], in_=p_[:, hs],
                                        pattern=[[1, 512]],
                                        compare_op=mybir.AluOpType.is_ge,
                                        fill=0.0,
                                        base=qT * 512 - kt * 128,
                                        channel_multiplier=-1)
                        for h in range(2):
                            kt = 2 * kp + h
                            hs = slice(h * 512, (h + 1) * 512)
                            nc.tensor.matmul(
                                o0[:], lhsT=v2[:, kt * 256:kt * 256 + 128],
                                rhs=p0[:, hs], start=(kt == 0),
                                stop=(kt == nkt - 1))
                            nc.tensor.matmul(
                                o1[:], lhsT=v2[:, kt * 256 + 128:(kt + 1) * 256],
                                rhs=p1[:, hs], start=(kt == 0),
                                stop=(kt == nkt - 1))
                    # normalize: rows 0:64 = O^T unnormalized, 64:128 = sums
                    # approx-recip is broken on base!=0 slices: run full-tile,
                    # consume rows 64:128 (the replicated sums) only
                    rc = pp.tile([128, 512], f32, tag="rc", bufs=2)
                    nc.vector.reciprocal_approx_fast(rc[:], o0[:])
                    nc.vector.tensor_tensor(
                        ao[0:64, qs], o0[0:64, :], rc[64:128, :],
                        mybir.AluOpType.mult)
                    rc2 = pp.tile([128, 512], f32, tag="rc", bufs=2)
                    nc.vector.reciprocal_approx_fast(rc2[:], o1[:])
                    nc.vector.tensor_tensor(
                        ao[64:128, qs], o1[0:64, :], rc2[64:128, :],
                        mybir.AluOpType.mult)

            # =========== Phase D: AllGather + o-proj ===========
            bounce = dram.tile([TP * 128, L], bf16)
            gathered = dram.tile([TP * TP * 128, L], bf16)
            # ao rows (g*64+d), free (jj*L + t) -> bounce row 256g+64jj+d
            for g in range(2):
                for jj in range(4):
                    nc.sync.dma_start(
                        bounce[256 * g + 64 * jj:256 * g + 64 * jj + 64, :],
                        ao[g * 64:(g + 1) * 64, jj * L:(jj + 1) * L])
            nc.gpsimd.collective_compute(
                "AllGather", mybir.AluOpType.bypass,
                replica_groups=[[0, 1, 2, 3], [4, 5, 6, 7]],
                ins=[bounce.opt()], outs=[gathered.opt()])

            with (
                tc.tile_pool(name="wo", bufs=1) as wop,
                tc.tile_pool(name="aog", bufs=24) as aogp,
                tc.tile_pool(name="ost", bufs=2) as ostp,
            ):
                wo_sb = wop.tile([128, CCH * 512], bf16)
                nc.sync.dma_start(
                    wo_sb[:].rearrange("p (c m) -> p c m", m=512),
                    WoT[:].rearrange("(c p) m -> p c m", p=128))
                for tt in range(TT):
                    ts = slice(tt * 512, (tt + 1) * 512)
                    aoks = []
                    for c in range(CCH):
                        aok = aogp.tile([128, 512], bf16, tag="aok",
                                        name=f"aok{tt}_{c}")
                        nc.sync.dma_start(aok[:],
                                          gathered[c * 128:(c + 1) * 128, ts])
                        aoks.append(aok)
                    for ct in range(4):
                        pso = ps.tile([128, 512], f32, tag="mm")
                        for c in range(CCH):
                            nc.tensor.matmul(
                                pso[:],
                                lhsT=wo_sb[:, c * 512 + ct * 128:
                                           c * 512 + (ct + 1) * 128],
                                rhs=aoks[c][:], start=(c == 0),
                                stop=(c == CCH - 1))
                        ost = ostp.tile([128, 512], f32, tag="ost")
                        nc.scalar.copy(ost[:], pso[:])
                        nc.sync.dma_start(
                            out[ct * 128:(ct + 1) * 128, ts], ost[:])

            if dbg:
                src = {"qq": qq, "kk": kk, "v2": v2, "ao": ao,
                       "gathered": gathered, "bounce": bounce}[dbg]
                nc.sync.dma_start(dbg_t[:], src[:])

    nc.compile()
    return nc


def _host_prep(hidden_states, cos, sin, Wq, Wk, Wv, Wo):
    """Build the 8 per-core input maps (all host-side slicing/transposes)."""
    scale = float(D) ** -0.5
    # rope coefficient tables [128, L]: 4 groups of 32 rows (d 0:32 pattern)
    cosT = cos[:, :32].T.astype(np.float32)          # [32, L]
    sinT = sin[:, :32].T.astype(np.float32)
    c1 = np.tile(cosT, (4, 1))                       # [128, L]
    c2 = np.concatenate([-sinT, sinT, -sinT, sinT], axis=0)
    tables = {
        "C1q": (c1 * scale).astype(BF16), "C2q": (c2 * scale).astype(BF16),
        "C1k": c1.astype(BF16), "C2k": c2.astype(BF16),
    }
    xTb = [np.ascontiguousarray(hidden_states[b].T).astype(BF16)
           for b in range(B)]
    in_maps = []
    for i in range(N_CORES):
        b, r = divmod(i, TP)
        # Wq rows reordered: M-tile m = heads (8r+m, 8r+4+m)
        rows = []
        for m in range(4):
            rows.append(Wq[(8 * r + m) * D:(8 * r + m + 1) * D])
            rows.append(Wq[(8 * r + 4 + m) * D:(8 * r + 4 + m + 1) * D])
        WqT_i = np.ascontiguousarray(np.concatenate(rows, 0).T).astype(BF16)
        WkT_i = np.ascontiguousarray(
            Wk[2 * r * D:(2 * r + 2) * D].T).astype(BF16)
        WvT_i = np.ascontiguousarray(
            Wv[2 * r * D:(2 * r + 2) * D].T).astype(BF16)
        WoT_i = np.ascontiguousarray(
            Wo[CW * r:CW * (r + 1), :].T).astype(BF16)
        in_maps.append({
            "xT": xTb[b], "WqT": WqT_i, "WkT": WkT_i, "WvT": WvT_i,
            "WoT": WoT_i, **tables,
        })
    return in_maps


def kernel(hidden_states, cos, sin, Wq, Wk, Wv, Wo, _want_profile=False):
    from concourse.bass_utils import run_bass_kernel_spmd

    if "nc" not in _cache:
        _cache["nc"] = _build_graph()
    nc = _cache["nc"]
    in_maps = _host_prep(np.asarray(hidden_states), np.asarray(cos),
                         np.asarray(sin), np.asarray(Wq), np.asarray(Wk),
                         np.asarray(Wv), np.asarray(Wo))
    res = run_bass_kernel_spmd(nc, in_maps, list(range(N_CORES)),
                               trace=_want_profile)
    # assemble: core (b, r) holds out^T [512, L] = cols [512r, 512r+512) of b
    full = np.empty((B, L, HID), np.float32)
    for i in range(N_CORES):
        b, r = divmod(i, TP)
        full[b, :, CW * r:CW * (r + 1)] = res.results[i]["out"].T
    if _want_profile:
        return full, res
    return full


# revision 28
# speedup vs baseline: 1.4350x; 1.0590x over previous
"""Distributed GQA attention kernel for one TRN2 chip (8 NeuronCores).

Problem: B=2, L=2048, HID=2048, H=32 q-heads, HKV=8 kv-heads, D=64,
rotary embedding, causal softmax, o-proj.

Sharding: core i -> batch b=i//4, TP rank r=i%4.  Each core computes
8 q-heads / 2 kv-heads of its batch, all-gathers the attention outputs
(feature-major, bf16) within its 4-core TP group, then computes its
512 output columns of the o-proj.  Host assembles the full output.

All matmuls run in bf16 with fp32 PSUM accumulation.  Softmax skips the
row-max (logits are bounded ~|6| for these input scales) and obtains
row sums for free by appending a 64-wide ones block to V's stationary
operand; normalization is a DVE reciprocal + multiply.
"""

import sys

sys.path.insert(0, "/opt/trn_rl_repo")

import numpy as np
import ml_dtypes

B, L, HID = 2, 2048, 2048
H, HKV, D = 32, 8, 64
N_CORES = 8
TP = 4           # tensor-parallel group size
HL = 8           # q heads per core
CW = 512         # o-proj output columns per core
TT = 4           # t tiles of 512 over L
CCH = HID // 128 # contraction chunks (16)
BF16 = ml_dtypes.bfloat16

_cache = {}


def _build_graph(dbg=None):
    import concourse.bass as bass
    import concourse.tile as tile
    from concourse import bacc, mybir

    dt = mybir.dt
    f32, bf16 = dt.float32, dt.bfloat16

    nc = bacc.Bacc("TRN2", target_bir_lowering=False, debug=False,
                   num_devices=N_CORES)

    xT = nc.dram_tensor("xT", [HID, L], bf16, kind="ExternalInput")
    WqT = nc.dram_tensor("WqT", [HID, HL * D], bf16, kind="ExternalInput")
    WkT = nc.dram_tensor("WkT", [HID, 128], bf16, kind="ExternalInput")
    WvT = nc.dram_tensor("WvT", [HID, 128], bf16, kind="ExternalInput")
    WoT = nc.dram_tensor("WoT", [HID, CW], bf16, kind="ExternalInput")
    C1q = nc.dram_tensor("C1q", [128, L], bf16, kind="ExternalInput")
    C2q = nc.dram_tensor("C2q", [128, L], bf16, kind="ExternalInput")
    C1k = nc.dram_tensor("C1k", [128, L], bf16, kind="ExternalInput")
    C2k = nc.dram_tensor("C2k", [128, L], bf16, kind="ExternalInput")
    out = nc.dram_tensor("out", [CW, L], f32, kind="ExternalOutput")
    dbg_shapes = {"qq": [128, HL // 2 * L], "kk": [128, L],
                  "v2": [128, CCH * 256], "ao": [128, HL // 2 * L],
                  "gathered": [TP * TP * 128, L], "bounce": [TP * 128, L]}
    dbg_t = (nc.dram_tensor("dbg", dbg_shapes[dbg], bf16,
                            kind="ExternalOutput") if dbg else None)

    def bcast_m(ap2d, n):
        # [P, F] -> [P, n, F] with a step-0 middle dim (free-dim broadcast)
        return bass.AP(ap2d.tensor, ap2d.offset,
                       [ap2d.ap[0], [0, n], ap2d.ap[1]])

    with tile.TileContext(nc) as tc:
        with (
            tc.tile_pool(name="persist", bufs=1) as persist,
            tc.tile_pool(name="ps", bufs=2, space="PSUM") as ps,
            tc.tile_pool(name="psbig", bufs=2, space="PSUM") as psbig,
            tc.tile_pool(name="pp", bufs=3) as pp,
            tc.tile_pool(name="dram", bufs=1, space="DRAM") as dram,
        ):
            # ---- persistent SBUF tensors ----
            qq = persist.tile([128, HL // 2 * L], bf16)      # roped Q^T, 2MB
            kk = persist.tile([128, L], bf16)                # roped K^T (2 kv)
            v2t = persist.tile([128, L], bf16)               # V^T staging
            v2 = persist.tile([128, CCH * 256], bf16)        # [V|1|V|1] per kt
            ao = persist.tile([128, HL // 2 * L], bf16)      # attn out^T
            c1q = persist.tile([128, L], bf16)
            c2q = persist.tile([128, L], bf16)
            c1k = persist.tile([128, L], bf16)
            c2k = persist.tile([128, L], bf16)
            for t_sb, t_dr in ((c1q, C1q), (c2q, C2q), (c1k, C1k), (c2k, C2k)):
                nc.scalar.dma_start(t_sb[:], t_dr[:])

            # ones blocks of v2 (columns 64:128 and 192:256 of each kt group)
            for off in (64, 192):
                ones_view = bass.AP(v2.tensor, v2.offset + off,
                                    [v2.ap[0], [256, CCH], [1, 64]])
                nc.gpsimd.memset(ones_view, 1.0)

            # =========== Phase B: projections + RoPE ===========
            with (
                tc.tile_pool(name="wsb", bufs=1) as wsb,
                tc.tile_pool(name="xt", bufs=2) as xtp,
                tc.tile_pool(name="rope", bufs=2) as rope,
            ):
                wq_sb = wsb.tile([128, CCH * 512], bf16)
                wq_v = WqT[:].rearrange("(c p) m -> p c m", p=128)
                wq_s = wq_sb[:].rearrange("p (c m) -> p c m", m=512)
                for cb in range(4):
                    nc.scalar.dma_start(wq_s[:, 4 * cb:4 * (cb + 1)],
                                        wq_v[:, 4 * cb:4 * (cb + 1)])
                wk_sb = wsb.tile([128, CCH * 128], bf16)
                nc.scalar.dma_start(
                    wk_sb[:].rearrange("p (c m) -> p c m", m=128),
                    WkT[:].rearrange("(c p) m -> p c m", p=128))
                wv_sb = wsb.tile([128, CCH * 128], bf16)
                nc.scalar.dma_start(
                    wv_sb[:].rearrange("p (c m) -> p c m", m=128),
                    WvT[:].rearrange("(c p) m -> p c m", p=128))

                xT_view = xT[:].rearrange("(c p) t -> p c t", p=128)

                for tt in range(TT):
                    ts = slice(tt * 512, (tt + 1) * 512)
                    xt = xtp.tile([128, CCH * 512], bf16, tag="xt")
                    xt_s = xt[:].rearrange("p (c t) -> p c t", t=512)
                    for cb in range(4):
                        nc.sync.dma_start(xt_s[:, 4 * cb:4 * (cb + 1)],
                                          xT_view[:, 4 * cb:4 * (cb + 1), ts])

                    # --- Q: 4 M-tiles (head pair (jj, jj+4) each) ---
                    qraw = rope.tile([128, 4 * 512], bf16, tag="qraw")
                    for m in range(4):
                        psq = ps.tile([128, 512], f32, tag="mm")
                        for c in range(CCH):
                            nc.tensor.matmul(
                                psq[:],
                                lhsT=wq_sb[:, c * 512 + m * 128:
                                           c * 512 + (m + 1) * 128],
                                rhs=xt[:, c * 512:(c + 1) * 512],
                                start=(c == 0), stop=(c == CCH - 1))
                        nc.scalar.copy(qraw[:, m * 512:(m + 1) * 512], psq[:])

                    # --- K ---
                    kraw = rope.tile([128, 512], bf16, tag="kraw")
                    psk = ps.tile([128, 512], f32, tag="mm")
                    for c in range(CCH):
                        nc.tensor.matmul(
                            psk[:], lhsT=wk_sb[:, c * 128:(c + 1) * 128],
                            rhs=xt[:, c * 512:(c + 1) * 512],
                            start=(c == 0), stop=(c == CCH - 1))
                    nc.scalar.copy(kraw[:], psk[:])

                    # --- V ---
                    psv = ps.tile([128, 512], f32, tag="mm")
                    for c in range(CCH):
                        nc.tensor.matmul(
                            psv[:], lhsT=wv_sb[:, c * 128:(c + 1) * 128],
                            rhs=xt[:, c * 512:(c + 1) * 512],
                            start=(c == 0), stop=(c == CCH - 1))
                    nc.scalar.copy(v2t[:, ts], psv[:])

                    # --- RoPE on Q ---
                    qsw = rope.tile([128, 4 * 512], bf16, tag="qsw")
                    for a, b_ in ((0, 32), (32, 0), (64, 96), (96, 64)):
                        nc.scalar.dma_start(qsw[b_:b_ + 32, :], qraw[a:a + 32, :])
                    qtmp = rope.tile([128, 4 * 512], bf16, tag="qtmp")
                    q3 = qraw[:].rearrange("p (m t) -> p m t", t=512)
                    s3 = qsw[:].rearrange("p (m t) -> p m t", t=512)
                    t3 = qtmp[:].rearrange("p (m t) -> p m t", t=512)
                    nc.vector.tensor_tensor(t3, q3, bcast_m(c1q[:, ts], 4),
                                            mybir.AluOpType.mult)
                    nc.vector.tensor_tensor(s3, s3, bcast_m(c2q[:, ts], 4),
                                            mybir.AluOpType.mult)
                    qqd = bass.AP(qq.tensor, qq.offset + tt * 512,
                                  [qq.ap[0], [2048, 4], [1, 512]])
                    nc.vector.tensor_tensor(qqd, t3, s3, mybir.AluOpType.add)

                    # --- RoPE on K ---
                    ksw = rope.tile([128, 512], bf16, tag="ksw")
                    for a, b_ in ((0, 32), (32, 0), (64, 96), (96, 64)):
                        nc.scalar.dma_start(ksw[b_:b_ + 32, :], kraw[a:a + 32, :])
                    ktmp = rope.tile([128, 512], bf16, tag="ktmp")
                    nc.vector.tensor_tensor(ktmp[:], kraw[:], c1k[:, ts],
                                            mybir.AluOpType.mult)
                    nc.vector.tensor_tensor(ksw[:], ksw[:], c2k[:, ts],
                                            mybir.AluOpType.mult)
                    nc.vector.tensor_tensor(kk[:, ts], ktmp[:], ksw[:],
                                            mybir.AluOpType.add)

                    # --- V transpose to token-major (v2 cols 0:64/128:192),
                    # per-tt so attention can start before proj finishes
                    for g in range(2):
                        v2_dst = bass.AP(v2.tensor,
                                         v2.offset + (4 * tt) * 256 + g * 128,
                                         [v2.ap[0], [256, 4], [1, 64]])
                        nc.sync.dma_start_transpose(
                            v2_dst, v2t[g * 64:(g + 1) * 64, ts])

            # =========== Phase C: attention ===========
            for jj in range(4):
                qoff = jj * L
                for qT in range(4):
                    nkt = 4 * qT + 4
                    qs = slice(qoff + qT * 512, qoff + (qT + 1) * 512)
                    o0 = ps.tile([128, 512], f32, tag="o")
                    o1 = ps.tile([128, 512], f32, tag="o")
                    for kp in range(nkt // 2):
                        sb0 = psbig.tile([128, 1024], f32, tag="s")
                        sb1 = psbig.tile([128, 1024], f32, tag="s")
                        for h in range(2):
                            kt = 2 * kp + h
                            kslc = slice(kt * 128, (kt + 1) * 128)
                            hs = slice(h * 512, (h + 1) * 512)
                            nc.tensor.matmul(
                                sb0[:, hs], lhsT=kk[0:64, kslc],
                                rhs=qq[0:64, qs], start=True, stop=True,
                                tile_position=(0, 0))
                            nc.tensor.matmul(
                                sb1[:, hs], lhsT=kk[64:128, kslc],
                                rhs=qq[64:128, qs], start=True, stop=True,
                                tile_position=(64, 0))
                        p0 = pp.tile([128, 1024], bf16, tag="p")
                        p1 = pp.tile([128, 1024], bf16, tag="p")
                        nc.scalar.activation(p0[:], sb0[:],
                                             mybir.ActivationFunctionType.Exp)
                        nc.scalar.activation(p1[:], sb1[:],
                                             mybir.ActivationFunctionType.Exp)
                        for h in range(2):
                            kt = 2 * kp + h
                            if kt >= 4 * qT:  # diagonal/above: causal mask
                                hs = slice(h * 512, (h + 1) * 512)
                                for p_ in (p0, p1):
                                    nc.gpsimd.affine_select(
                                        out=p_[:, hs], in_=p_[:, hs],
                                        pattern=[[1, 512]],
                                        compare_op=mybir.AluOpType.is_ge,
                                        fill=0.0,
                                        base=qT * 512 - kt * 128,
                                        channel_multiplier=-1)
                        for h in range(2):
                            kt = 2 * kp + h
                            hs = slice(h * 512, (h + 1) * 512)
                            nc.tensor.matmul(
                                o0[:], lhsT=v2[:, kt * 256:kt * 256 + 128],
                                rhs=p0[:, hs], start=(kt == 0),
                                stop=(kt == nkt - 1))
                            nc.tensor.matmul(
                                o1[:], lhsT=v2[:, kt * 256 + 128:(kt + 1) * 256],
                                rhs=p1[:, hs], start=(kt == 0),
                                stop=(kt == nkt - 1))
                    # normalize: rows 0:64 = O^T unnormalized, 64:128 = sums
                    # approx-recip is broken on base!=0 slices: run full-tile,
                    # consume rows 64:128 (the replicated sums) only
                    rc = pp.tile([128, 512], f32, tag="rc", bufs=2)
                    nc.vector.reciprocal_approx_fast(rc[:], o0[:])
                    nc.vector.tensor_tensor(
                        ao[0:64, qs], o0[0:64, :], rc[64:128, :],
                        mybir.AluOpType.mult)
                    rc2 = pp.tile([128, 512], f32, tag="rc", bufs=2)
                    nc.vector.reciprocal_approx_fast(rc2[:], o1[:])
                    nc.vector.tensor_tensor(
                        ao[64:128, qs], o1[0:64, :], rc2[64:128, :],
                        mybir.AluOpType.mult)

            # =========== Phase D: split AllGather + o-proj ===========
            # AG half h covers heads jj in {2h, 2h+1}; issued as soon as the
            # attention loop (jj-major) finishes those heads, so AG overlaps
            # the remaining attention and the first o-proj half overlaps AG2.
            # bounce_h row layout: 128*g + 64*(jj-2h) + d.
            bounces = [dram.tile([2 * 128, L], bf16, name=f"bounce{h}")
                       for h in range(2)]
            gath = [dram.tile([TP * 2 * 128, L], bf16, name=f"gath{h}")
                    for h in range(2)]
            for h in range(2):
                for g in range(2):
                    for jj in (2 * h, 2 * h + 1):
                        nc.sync.dma_start(
                            bounces[h][128 * g + 64 * (jj - 2 * h):
                                       128 * g + 64 * (jj - 2 * h) + 64, :],
                            ao[g * 64:(g + 1) * 64, jj * L:(jj + 1) * L])
                nc.gpsimd.collective_compute(
                    "AllGather", mybir.AluOpType.bypass,
                    replica_groups=[[0, 1, 2, 3], [4, 5, 6, 7]],
                    ins=[bounces[h].opt()], outs=[gath[h].opt()])

            with (
                tc.tile_pool(name="wo", bufs=1) as wop,
                tc.tile_pool(name="aog", bufs=24) as aogp,
                tc.tile_pool(name="ost", bufs=2) as ostp,
            ):
                wo_sb = wop.tile([128, CCH * 512], bf16)
                nc.sync.dma_start(
                    wo_sb[:].rearrange("p (c m) -> p c m", m=512),
                    WoT[:].rearrange("(c p) m -> p c m", p=128))
                for tt in range(TT):
                    ts = slice(tt * 512, (tt + 1) * 512)
                    aoks = []
                    for c in range(CCH):
                        aok = aogp.tile([128, 512], bf16, tag="aok",
                                        name=f"aok{tt}_{c}")
                        src = gath[c // 8]
                        nc.sync.dma_start(
                            aok[:], src[(c % 8) * 128:(c % 8 + 1) * 128, ts])
                        aoks.append(aok)
                    for ct in range(4):
                        pso = ps.tile([128, 512], f32, tag="mm")
                        for c in range(CCH):
                            nc.tensor.matmul(
                                pso[:],
                                lhsT=wo_sb[:, c * 512 + ct * 128:
                                           c * 512 + (ct + 1) * 128],
                                rhs=aoks[c][:], start=(c == 0),
                                stop=(c == CCH - 1))
                        ost = ostp.tile([128, 512], f32, tag="ost")
                        nc.scalar.copy(ost[:], pso[:])
                        nc.sync.dma_start(
                            out[ct * 128:(ct + 1) * 128, ts], ost[:])

            if dbg:
                src = {"qq": qq, "kk": kk, "v2": v2, "ao": ao}[dbg]
                nc.sync.dma_start(dbg_t[:], src[:])

    nc.compile()
    return nc


def _host_prep(hidden_states, cos, sin, Wq, Wk, Wv, Wo):
    """Build the 8 per-core input maps (all host-side slicing/transposes)."""
    scale = float(D) ** -0.5
    # rope coefficient tables [128, L]: 4 groups of 32 rows (d 0:32 pattern)
    cosT = cos[:, :32].T.astype(np.float32)          # [32, L]
    sinT = sin[:, :32].T.astype(np.float32)
    c1 = np.tile(cosT, (4, 1))                       # [128, L]
    c2 = np.concatenate([-sinT, sinT, -sinT, sinT], axis=0)
    tables = {
        "C1q": (c1 * scale).astype(BF16), "C2q": (c2 * scale).astype(BF16),
        "C1k": c1.astype(BF16), "C2k": c2.astype(BF16),
    }
    xTb = [np.ascontiguousarray(hidden_states[b].T).astype(BF16)
           for b in range(B)]
    in_maps = []
    for i in range(N_CORES):
        b, r = divmod(i, TP)
        # Wq rows reordered: M-tile m = heads (8r+m, 8r+4+m)
        rows = []
        for m in range(4):
            rows.append(Wq[(8 * r + m) * D:(8 * r + m + 1) * D])
            rows.append(Wq[(8 * r + 4 + m) * D:(8 * r + 4 + m + 1) * D])
        WqT_i = np.ascontiguousarray(np.concatenate(rows, 0).T).astype(BF16)
        WkT_i = np.ascontiguousarray(
            Wk[2 * r * D:(2 * r + 2) * D].T).astype(BF16)
        WvT_i = np.ascontiguousarray(
            Wv[2 * r * D:(2 * r + 2) * D].T).astype(BF16)
        # o-proj k-rows ordered to match the two gathered buffers:
        # half h row R: rank=R//256, g=(R%256)//128, jj=2h+(R%128)//64, d=R%64
        RR = np.arange(1024)
        perm = []
        for h in range(2):
            f = ((8 * (RR // 256) + 4 * ((RR % 256) // 128)
                  + 2 * h + (RR % 128) // 64) * D + RR % 64)
            perm.append(f)
        perm = np.concatenate(perm)
        WoT_i = np.ascontiguousarray(
            Wo[CW * r:CW * (r + 1), :].T[perm]).astype(BF16)
        in_maps.append({
            "xT": xTb[b], "WqT": WqT_i, "WkT": WkT_i, "WvT": WvT_i,
            "WoT": WoT_i, **tables,
        })
    return in_maps


def kernel(hidden_states, cos, sin, Wq, Wk, Wv, Wo, _want_profile=False):
    from concourse.bass_utils import run_bass_kernel_spmd

    if "nc" not in _cache:
        _cache["nc"] = _build_graph()
    nc = _cache["nc"]
    in_maps = _host_prep(np.asarray(hidden_states), np.asarray(cos),
                         np.asarray(sin), np.asarray(Wq), np.asarray(Wk),
                         np.asarray(Wv), np.asarray(Wo))
    res = run_bass_kernel_spmd(nc, in_maps, list(range(N_CORES)),
                               trace=_want_profile)
    # assemble: core (b, r) holds out^T [512, L] = cols [512r, 512r+512) of b
    full = np.empty((B, L, HID), np.float32)
    for i in range(N_CORES):
        b, r = divmod(i, TP)
        full[b, :, CW * r:CW * (r + 1)] = res.results[i]["out"].T
    if _want_profile:
        return full, res
    return full
